# revision 8
# baseline (speedup 1.0000x reference)
"""Trainium2 Bass kernel for nn_DMF_50130858279442.

Reference computation (M=100 Monte-Carlo mutations, fixed RNG key 42):
    std_h[b]   = std(f_h[b,:], ddof=1)                     (per half h)
    G_h        = mask * normal_draw            [M,B,D]     (input-INDEPENDENT)
    cand       = f_h[None] + G_h * std_h                   [M,B,D]
    var[b,d]   = var_m(cand) + eps  = std_h[b]^2 * var_m(G_h)[b,d] + eps
    w          = (1/var) / sum_b(1/var)
    Z[b,d]     = sum_m(cand*w) = w * M * (f_h[b,d] + std_h[b]*mean_m(G_h)[b,d])

Because the RNG key is fixed, A = mean_m(G) and V = var_m(G) are constants
([B,D] per half) precomputed once on host CPU. The device kernel computes the
closed form.

Sharding: over the 2D=1024 OUTPUT COLUMNS (128 per core) — cores 0-3 own the
spatial half's columns, 4-7 the spectral half's. The batch-axis normalization
sum is then core-local (no collective; an AllReduce here pays a ~47us
cross-core launch-skew barrier on this stack). Each core redundantly computes
the row-std of its own half (needs the full [1024,512] half, 2 MiB extra DMA)
— that trade wins by a wide margin.

Per-core layout trick: the 1024 batch rows are folded into 8 "row blocks" of
128 (SBUF partition dim); block k lives at free-axis offset 128*k (or 512*k
for f). Row b=128k+p sits at [partition p, block k].
"""

import numpy as np

P = 128          # SBUF partitions = rows per block
KB = 8           # row blocks (B = KB*P)
D = 512          # per-half feature dim
W = 128          # output columns per core
NCORES = 8
M_MUT = 100      # NUM_MUTATIONS
RATE = 0.2       # MUTATION_RATE
EPS = 1e-6
B = 1024

_CACHE = {}


def _gstats():
    """A = mean_m(mask*normal), Vp = var_m(mask*normal)/(D-1) for both halves.

    Computed once on host CPU with the reference's exact RNG (jax threefry,
    key 42). Vp folds the 1/(D-1) of the unbiased row-variance so the device
    computes var_cand = u * Vp with u = sumsq - sum^2/D (= (D-1)*std^2).
    """
    if "gstats" in _CACHE:
        return _CACHE["gstats"]
    import jax
    import jax.numpy as jnp

    cpu = jax.devices("cpu")[0]
    out = {}
    with jax.default_device(cpu):
        rng = jax.random.key(42)
        km1, kn1, km2, kn2 = jax.random.split(rng, 4)
        for name, km, kn in (("sp", km1, kn1), ("spec", km2, kn2)):
            mask = np.asarray(
                jax.random.uniform(km, (M_MUT, B, D), dtype=jnp.float32) < RATE
            )
            noise = np.asarray(
                jax.random.normal(kn, (M_MUT, B, D), dtype=jnp.float32)
            )
            G = np.where(mask, noise, np.float32(0.0)).astype(np.float64)
            A = G.mean(axis=0)
            V = G.var(axis=0)  # ddof=0, matches jnp.var over mutations
            out[name] = (
                np.ascontiguousarray(A, dtype=np.float32),
                np.ascontiguousarray(V / (D - 1), dtype=np.float32),
            )
    _CACHE["gstats"] = out
    return out


def _build_bass():
    """Per-core SPMD Tile kernel (identical program; per-core data differs)."""
    if "nc" in _CACHE:
        return _CACHE["nc"]
    import concourse.bacc as bacc
    import concourse.mybir as mybir
    from concourse import tile

    f32 = mybir.dt.float32
    AF = mybir.ActivationFunctionType
    OP = mybir.AluOpType

    nc = bacc.Bacc(
        "TRN2", target_bir_lowering=False, debug=False, num_devices=NCORES
    )

    # f_blk: the core's FULL half, block-major [KB, P, D] (for row stats)
    f_blk = nc.dram_tensor("f_blk", [KB, P, D], f32, kind="ExternalInput")
    # column slices for this core's W output columns, block-concat [P, KB*W]
    fc_d = nc.dram_tensor("fc", [P, KB * W], f32, kind="ExternalInput")
    a_d = nc.dram_tensor("ac", [P, KB * W], f32, kind="ExternalInput")
    v_d = nc.dram_tensor("vc", [P, KB * W], f32, kind="ExternalInput")
    z_d = nc.dram_tensor("z", [P, KB * W], f32, kind="ExternalOutput")

    with tile.TileContext(nc) as tc:
        with (
            tc.tile_pool(name="pool", bufs=1) as pool,
            tc.tile_pool(name="psum", bufs=1, space="PSUM") as psum,
        ):
            ones_col = pool.tile([P, 1], f32)
            nc.vector.memset(ones_col[:], 1.0)
            # value M_MUT: folds the *M of the closed form into the
            # K=1 broadcast matmul bc = m_row.T @ r
            m_row = pool.tile([1, W], f32)
            nc.vector.memset(m_row[:], float(M_MUT))

            fc = pool.tile([P, KB * W], f32)
            a = pool.tile([P, KB * W], f32)
            v = pool.tile([P, KB * W], f32)
            nc.sync.dma_start(fc[:], fc_d[:])
            nc.sync.dma_start(a[:], a_d[:])
            nc.sync.dma_start(v[:], v_d[:])

            # ---- row stats (redundant per core, for its half) ----
            sumsq = pool.tile([P, KB], f32)
            sumf = pool.tile([P, KB], f32)
            fks = []
            for k in range(KB):
                fk = pool.tile([P, D], f32, name=f"fk{k}", tag="fk", bufs=KB)
                nc.sync.dma_start(fk[:], f_blk[k])
                fks.append(fk)
            for k in range(KB):
                sqk = pool.tile([P, D], f32, name=f"sq{k}", tag="sq", bufs=2)
                nc.scalar.activation(
                    sqk[:], fks[k][:], AF.Square, accum_out=sumsq[:, k : k + 1]
                )
                nc.vector.reduce_sum(
                    out=sumf[:, k : k + 1],
                    in_=fks[k][:],
                    axis=mybir.AxisListType.X,
                )

            # u = sumsq - sumf^2/D = (D-1)*std^2 ;  s = sqrt(u/(D-1))
            t1 = pool.tile([P, KB], f32)
            nc.vector.tensor_mul(t1[:], sumf[:], sumf[:])
            u = pool.tile([P, KB], f32)
            nc.vector.scalar_tensor_tensor(
                out=u[:],
                in0=t1[:],
                scalar=-1.0 / D,
                in1=sumsq[:],
                op0=OP.mult,
                op1=OP.add,
            )
            s = pool.tile([P, KB], f32)
            nc.scalar.activation(s[:], u[:], AF.Sqrt, scale=1.0 / (D - 1))

            # ---- per-block column phase ----
            # t = u_k * Vp + eps ; invw = 1/t (one fused approx recip)
            t = pool.tile([P, KB * W], f32)
            for k in range(KB):
                c0 = k * W
                nc.vector.tensor_scalar(
                    out=t[:, c0 : c0 + W],
                    in0=v[:, c0 : c0 + W],
                    scalar1=u[:, k : k + 1],
                    scalar2=float(EPS),
                    op0=OP.mult,
                    op1=OP.add,
                )
            invw = pool.tile([P, KB * W], f32)
            nc.vector.reciprocal_approx_fast(invw[:], t[:])

            # colsum over all B rows: accumulate 8 block matmuls in PSUM
            part = psum.tile([1, W], f32)
            for k in range(KB):
                c0 = k * W
                nc.tensor.matmul(
                    part[:],
                    ones_col[:],
                    invw[:, c0 : c0 + W],
                    start=(k == 0),
                    stop=(k == KB - 1),
                )
            cs = pool.tile([1, W], f32)
            nc.vector.tensor_copy(cs[:], part[:])
            r = pool.tile([1, W], f32)
            nc.vector.reciprocal_approx_fast(r[:], cs[:])
            # broadcast M/colsum over partitions via K=1 matmul
            bc = psum.tile([P, W], f32)
            nc.tensor.matmul(bc[:], m_row[:], r[:])

            # numer = (A*s_k + f) * invw ; z = numer * bc
            numer = pool.tile([P, KB * W], f32)
            for k in range(KB):
                c0 = k * W
                nc.vector.scalar_tensor_tensor(
                    out=numer[:, c0 : c0 + W],
                    in0=a[:, c0 : c0 + W],
                    scalar=s[:, k : k + 1],
                    in1=fc[:, c0 : c0 + W],
                    op0=OP.mult,
                    op1=OP.add,
                )
            numer2 = pool.tile([P, KB * W], f32)
            nc.vector.tensor_mul(numer2[:], numer[:], invw[:])
            z = pool.tile([P, KB * W], f32)
            for k in range(KB):
                c0 = k * W
                nc.vector.tensor_mul(
                    z[:, c0 : c0 + W], numer2[:, c0 : c0 + W], bc[:]
                )
            nc.sync.dma_start(z_d[:], z[:])

    nc.compile()
    _CACHE["nc"] = nc
    return nc


def _blockify(x):
    """[B, W] -> [P, KB*W]: row 128k+p lands at [p, k*W:(k+1)*W]."""
    Bx, Wx = x.shape
    return np.ascontiguousarray(
        x.reshape(KB, P, Wx).transpose(1, 0, 2).reshape(P, KB * Wx)
    )


def _unblockify(x):
    """[P, KB*W] -> [B, W] (inverse of _blockify)."""
    Px, KW = x.shape
    Wx = KW // KB
    return x.reshape(P, KB, Wx).transpose(1, 0, 2).reshape(KB * P, Wx)


def _in_maps(spatial, spectral):
    g = _gstats()
    halves = {"sp": (spatial, *g["sp"]), "spec": (spectral, *g["spec"])}
    maps = []
    for c in range(NCORES):
        half = "sp" if c < 4 else "spec"
        f_h, A_h, Vp_h = halves[half]
        ccol = (c % 4) * W
        maps.append(
            {
                "f_blk": np.ascontiguousarray(f_h.reshape(KB, P, D)),
                "fc": _blockify(f_h[:, ccol : ccol + W]),
                "ac": _blockify(A_h[:, ccol : ccol + W]),
                "vc": _blockify(Vp_h[:, ccol : ccol + W]),
            }
        )
    return maps


def run(spatial_features, spectral_features, trace=False, **kwargs):
    """Run the SPMD bass kernel; returns (Z [1024,1024] f32, BassKernelResults)."""
    from concourse.bass_utils import run_bass_kernel_spmd

    spatial = np.ascontiguousarray(np.asarray(spatial_features, dtype=np.float32))
    spectral = np.ascontiguousarray(np.asarray(spectral_features, dtype=np.float32))
    assert spatial.shape == (B, D) and spectral.shape == (B, D)

    nc = _build_bass()
    res = run_bass_kernel_spmd(
        nc,
        _in_maps(spatial, spectral),
        core_ids=list(range(NCORES)),
        trace=trace,
        **kwargs,
    )
    z = np.empty((B, 2 * D), dtype=np.float32)
    for c in range(NCORES):
        z[:, c * W : (c + 1) * W] = _unblockify(res.results[c]["z"])
    return z, res


def kernel(spatial_features, spectral_features):
    z, _ = run(spatial_features, spectral_features, trace=False)
    return z
